# revision 17
# baseline (speedup 1.0000x reference)
"""CrossCosineEmbeddingLoss kernel for 8 trn2 NeuronCores (v10).

loss = mean over all (i,j) of: 1 - cos(x_i, y_j) if i==j else relu(cos(x_i, y_j))

Identity:  total = sum_ij relu(sim_ij) + sum_i (1 - sim_ii - relu(sim_ii))

Sharding (2x4 grid): core c = (bi, bj), bi = c // 2, bj = c % 2.
  x rows [2048*bi, 2048*(bi+1)) x y rows [4096*bj, 4096*(bj+1)).
Each core computes sum_ij relu(x_hat_i . y_j) / ||y_j|| over its block.
Diag correction only used from cores whose x block lies in their y range.

Scheduling notes:
 - GpSimd compute holds the shared SBUF port and blocks DVE 2-port ops,
   so GpSimd only generates SWDGE descriptors in the head; y squares run
   on DVE as single-port 2x_1P tensor_tensor ops in 4 small chunks.
 - x loads via HWDGE (earlier data), squares on ACT (Square big-op).
 - rny is produced per 8-tile quarter so early reduce tiles never wait
   on later chunks; later quarter tails are emitted inside the loop.

Per-core pipeline:
  - x: 2 HWDGE fp32 half-DMAs (row = 16p+8u+t); ACT Square; DVE reduce;
    rsqrt; DVE tensor_scalar scale-cast bf16; PE transpose; ACT copy
  - y: 4 SWDGE cast-DMA chunks; DVE square+reduce+recip / ACT sqrt per
    quarter; PE transpose + DVE copy -> yT
  - yd: 2 SWDGE cast-DMAs (same layout as x), only feeds diag
  - main: 32 j-tiles of [128j, 2048i] fp32 PSUM; per tile two 2-bank
    pool tiles (bufs=2 each): ACT activation(Relu, scale=rny, accum_out)
    / DVE tensor_scalar(max 0, accum_out) -> R, outputs to SBUF dumps
  - diag: GpSimd squares mid-loop + late DVE reduces, tiny fp32 ops
Host combines [128,2] partials; diag col used only from owner cores.
"""

import numpy as np

import concourse.bacc as bacc
import concourse.bass as bass
import concourse.tile as tile
from concourse import mybir
from concourse.bass_utils import run_bass_kernel_spmd
from concourse.masks import make_identity

N, D = 8192, 128
NCORES = 8
XI = 2048            # x rows per core
YJ = 4096            # y rows per core
TXI = XI // 128      # 16 x tiles
TYJ = YJ // 128      # 32 y j-tiles
YCH = 4              # y cast-DMA chunks
YCT = TYJ // YCH     # 8 j-tiles per chunk
ACW = 1024           # ACT's share of each [128, 2048] tile (bank aligned)

f32 = mybir.dt.float32
bf16 = mybir.dt.bfloat16
AF = mybir.ActivationFunctionType
ALU = mybir.AluOpType
AX = mybir.AxisListType

_CACHE = {}


def _build():
    if "nc" in _CACHE:
        return _CACHE["nc"]
    nc = bacc.Bacc("TRN2", target_bir_lowering=False, debug=False,
                   num_devices=NCORES)
    xs_d = nc.dram_tensor("xs", [XI, D], f32, kind="ExternalInput")
    y_d = nc.dram_tensor("y", [YJ, D], f32, kind="ExternalInput")
    yd_d = nc.dram_tensor("yd", [XI, D], f32, kind="ExternalInput")
    out_d = nc.dram_tensor("out", [128, 2], f32, kind="ExternalOutput")

    with tile.TileContext(nc) as tc:
        with (
            tc.tile_pool(name="singles", bufs=1) as singles,
            tc.tile_pool(name="scr", bufs=2) as scr,
        ):
            ident = singles.tile([128, 128], bf16)
            make_identity(nc, ident[:])
            warm = singles.tile([128, 1], f32)
            nc.vector.memset(warm[:], 1.0)
            nc.scalar.sqrt(warm[:], warm[:])   # preload sqrt table set early

            ynat = singles.tile([128, TYJ, 128], bf16)   # row j: 1024g+16p+t
            yT = singles.tile([128, TYJ, 128], bf16)     # [d, t, j-col]
            xnat = singles.tile([128, TXI, 128], f32)    # row i: 16p+8u+t
            xhat = singles.tile([128, TXI, 128], bf16)
            xhatT = singles.tile([128, TXI * 128], bf16)
            ydn = singles.tile([128, TXI, 128], bf16)    # same layout as x

            ny2 = singles.tile([128, TYJ], f32)
            t2y = singles.tile([128, TYJ], f32)
            rny = singles.tile([128, TYJ], f32)
            nx2 = singles.tile([128, TXI], f32)
            t1x = singles.tile([128, TXI], f32)
            rnx = singles.tile([128, TXI], f32)
            d2 = singles.tile([128, TXI], f32)
            nyd2 = singles.tile([128, TXI], f32)
            rnyd = singles.tile([128, TXI], f32)
            sim_d = singles.tile([128, TXI], f32)
            relu_d = singles.tile([128, TXI], f32)
            R = singles.tile([128, 64], f32)
            outsb = singles.tile([128, 2], f32)
            dumpA = singles.tile([128, ACW], bf16)
            dumpD = singles.tile([128, 2048 - ACW], bf16)

            # ---- input DMAs: x on HWDGE (fp32); y, yd on SWDGE (cast)
            xv = xs_d[:].rearrange("(p u t) d -> u p t d", u=2, t=8)
            for h in range(2):
                nc.sync.dma_start(
                    out=xnat[:, 8 * h:8 * (h + 1), :],
                    in_=xv[h:h + 1].rearrange("u p t d -> (u p) t d"))
            for g in range(YCH):
                rows = slice(1024 * g, 1024 * (g + 1))
                nc.gpsimd.dma_start(
                    out=ynat[:, YCT * g:YCT * (g + 1), :],
                    in_=y_d[rows].rearrange("(p t) d -> p t d", t=YCT))
            ydv = yd_d[:].rearrange("(p u t) d -> u p t d", u=2, t=8)
            for h in range(2):
                nc.gpsimd.dma_start(
                    out=ydn[:, 8 * h:8 * (h + 1), :],
                    in_=ydv[h:h + 1].rearrange("u p t d -> (u p) t d"))

            # ---- x norms + scale per half: ACT square, DVE reduce/scale
            for h in range(2):
                hs = slice(8 * h, 8 * (h + 1))
                xsq = scr.tile([128, 8, 128], bf16, tag="xsq",
                               name=f"xsq{h}")
                with nc.allow_low_precision("norm squares in bf16"):
                    nc.scalar.activation(
                        xsq[:].rearrange("p a b -> p (a b)"),
                        xnat[:, hs, :].rearrange("p a b -> p (a b)"),
                        AF.Square)
                nc.vector.tensor_reduce(out=nx2[:, hs], in_=xsq[:],
                                        axis=AX.X, op=ALU.add)
                nc.vector.reciprocal(t1x[:, hs], nx2[:, hs])
                nc.scalar.sqrt(rnx[:, hs], t1x[:, hs])   # 1/||x_i||
                for t in range(8 * h, 8 * h + 8):
                    nc.vector.tensor_scalar(
                        out=xhat[:, t, :], in0=xnat[:, t, :],
                        scalar1=rnx[:, t:t + 1], scalar2=None,
                        op0=ALU.mult)

            # ---- y norms per quarter: DVE single-port square + reduce
            def y_norm_tail(g):
                gs = slice(YCT * g, YCT * (g + 1))
                with nc.allow_low_precision("norm squares in bf16"):
                    s = scr.tile([128, YCT, 128], bf16, tag="ysq",
                                 name=f"ysq{g}")
                    nc.vector.tensor_mul(
                        s[:].rearrange("p a b -> p (a b)"),
                        ynat[:, gs, :].rearrange("p a b -> p (a b)"),
                        ynat[:, gs, :].rearrange("p a b -> p (a b)"))
                nc.vector.tensor_reduce(out=ny2[:, gs], in_=s[:],
                                        axis=AX.X, op=ALU.add)
                nc.vector.reciprocal(t2y[:, gs], ny2[:, gs])
                nc.scalar.sqrt(rny[:, gs], t2y[:, gs])   # 1/||y_j||

            y_norm_tail(0)
            y_norm_tail(1)

            # ---- transposes on PE (bf16) + copies to SBUF
            with tc.tile_pool(name="tpsum", bufs=2, space="PSUM") as tpsum:
                for h in range(2):
                    ptx = tpsum.tile([128, 1024], bf16, tag="tp2")
                    for k in range(8):
                        t = 8 * h + k
                        nc.tensor.transpose(ptx[:, 128 * k:128 * (k + 1)],
                                            xhat[:, t, :], ident[:])
                    nc.scalar.copy(out=xhatT[:, 1024 * h:1024 * (h + 1)],
                                   in_=ptx[:])
                for g in range(YCH):
                    pty = tpsum.tile([128, 1024], bf16, tag="tp2")
                    for k in range(YCT):
                        t = YCT * g + k
                        nc.tensor.transpose(pty[:, 128 * k:128 * (k + 1)],
                                            ynat[:, t, :], ident[:])
                    nc.vector.tensor_copy(
                        out=yT[:, YCT * g:YCT * (g + 1), :]
                        .rearrange("p a b -> p (a b)"),
                        in_=pty[:])

            # ---- main loop
            with (
                tc.tile_pool(name="mpa", bufs=2, space="PSUM") as mpa,
                tc.tile_pool(name="mpd", bufs=2, space="PSUM") as mpd,
            ):
                prodn = ydsqn = None
                for t in range(TYJ):
                    if t == 2:
                        y_norm_tail(2)
                    if t == 6:
                        # diag squares on GpSimd (idle mid-loop, SBUF only)
                        prodn = scr.tile([128, TXI, 128], bf16, tag="gp",
                                         name="prodn")
                        nc.gpsimd.tensor_mul(
                            prodn[:].rearrange("p a b -> p (a b)"),
                            xhat[:].rearrange("p a b -> p (a b)"),
                            ydn[:].rearrange("p a b -> p (a b)"))
                        ydsqn = scr.tile([128, TXI, 128], bf16, tag="gq",
                                         name="ydsqn")
                        nc.gpsimd.tensor_mul(
                            ydsqn[:].rearrange("p a b -> p (a b)"),
                            ydn[:].rearrange("p a b -> p (a b)"),
                            ydn[:].rearrange("p a b -> p (a b)"))
                    if t == 10:
                        y_norm_tail(3)
                    if t == 24:
                        nc.vector.tensor_reduce(out=d2[:], in_=prodn[:],
                                                axis=AX.X, op=ALU.add)
                    if t == 28:
                        nc.vector.tensor_reduce(out=nyd2[:], in_=ydsqn[:],
                                                axis=AX.X, op=ALU.add)

                    lhsT = yT[:, t, :]
                    pa = mpa.tile([128, ACW], f32, tag="pa")
                    pd = mpd.tile([128, 2048 - ACW], f32, tag="pd")
                    for k in range(4):
                        col = 512 * k
                        dst = (pa[:, col:col + 512] if col < ACW
                               else pd[:, col - ACW:col - ACW + 512])
                        nc.tensor.matmul(dst, lhsT,
                                         xhatT[:, col:col + 512])
                    nc.scalar.activation(
                        dumpA[:], pa[:], AF.Relu,
                        scale=rny[:, t:t + 1],
                        accum_out=R[:, 2 * t:2 * t + 1])
                    nc.vector.tensor_scalar(
                        out=dumpD[:], in0=pd[:],
                        scalar1=0.0, scalar2=None,
                        op0=ALU.max, op1=ALU.add,
                        accum_out=R[:, 2 * t + 1:2 * t + 2])

            # post-scale DVE R columns (odd) by rny
            nc.vector.tensor_mul(R[:, 1:64:2], R[:, 1:64:2], rny[:, 0:TYJ])

            # ---- diag scalars
            nc.vector.reciprocal(t1x[:], nyd2[:])
            nc.scalar.sqrt(rnyd[:], t1x[:])
            nc.vector.tensor_mul(sim_d[:], d2[:], rnyd[:])
            nc.scalar.activation(relu_d[:], sim_d[:], AF.Relu)
            nc.vector.scalar_tensor_tensor(
                out=scr.tile([128, TXI], f32, tag="dd", name="dd")[:],
                in0=sim_d[:], scalar=1.0, in1=relu_d[:],
                op0=ALU.mult, op1=ALU.add, accum_out=outsb[:, 1:2])

            # ---- final: sum R columns
            nc.vector.tensor_reduce(out=outsb[:, 0:1], in_=R[:],
                                    axis=AX.X, op=ALU.add)
            nc.sync.dma_start(out=out_d[:], in_=outsb[:])

    nc.compile()
    _CACHE["nc"] = nc
    return nc


# cores whose x block lies inside their y range own the diag correction
_DIAG_OWNER = [1, 0, 1, 0, 0, 1, 0, 1]


def _in_maps(x, y):
    maps = []
    for c in range(NCORES):
        bi, bj = c // 2, c % 2
        xsl = slice(XI * bi, XI * (bi + 1))
        ysl = slice(YJ * bj, YJ * (bj + 1))
        maps.append({"xs": np.ascontiguousarray(x[xsl]),
                     "y": np.ascontiguousarray(y[ysl]),
                     "yd": np.ascontiguousarray(y[xsl])})
    return maps


def _combine(results):
    total = 0.0
    for c in range(NCORES):
        o = results[c]["out"].astype(np.float64)
        total += o[:, 0].sum()
        if _DIAG_OWNER[c]:
            total += XI - o[:, 1].sum()
    return np.float32(total / (float(N) * float(N)))


def _run(x, y, trace=False):
    nc = _build()
    res = run_bass_kernel_spmd(nc, _in_maps(x, y), list(range(NCORES)),
                               trace=trace)
    return _combine(res.results), res


def kernel(x, y):
    x = np.asarray(x, dtype=np.float32)
    y = np.asarray(y, dtype=np.float32)
    loss, _ = _run(x, y, trace=False)
    return loss


# revision 18
# speedup vs baseline: 1.0065x; 1.0065x over previous
"""CrossCosineEmbeddingLoss kernel for 8 trn2 NeuronCores (v10).

loss = mean over all (i,j) of: 1 - cos(x_i, y_j) if i==j else relu(cos(x_i, y_j))

Identity:  total = sum_ij relu(sim_ij) + sum_i (1 - sim_ii - relu(sim_ii))

Sharding (2x4 grid): core c = (bi, bj), bi = c // 2, bj = c % 2.
  x rows [2048*bi, 2048*(bi+1)) x y rows [4096*bj, 4096*(bj+1)).
Each core computes sum_ij relu(x_hat_i . y_j) / ||y_j|| over its block.
Diag correction only used from cores whose x block lies in their y range.

Scheduling notes:
 - GpSimd compute holds the shared SBUF port and blocks DVE 2-port ops,
   so GpSimd only generates SWDGE descriptors in the head; y squares run
   on DVE as single-port 2x_1P tensor_tensor ops in 4 small chunks.
 - x loads via HWDGE (earlier data), squares on ACT (Square big-op).
 - rny is produced per 8-tile quarter so early reduce tiles never wait
   on later chunks; later quarter tails are emitted inside the loop.

Per-core pipeline:
  - x: 2 HWDGE fp32 half-DMAs (row = 16p+8u+t); ACT Square; DVE reduce;
    rsqrt; DVE tensor_scalar scale-cast bf16; PE transpose; ACT copy
  - y: 4 SWDGE cast-DMA chunks; DVE square+reduce+recip / ACT sqrt per
    quarter; PE transpose + DVE copy -> yT
  - yd: 2 SWDGE cast-DMAs (same layout as x), only feeds diag
  - main: 32 j-tiles of [128j, 2048i] fp32 PSUM; per tile two 2-bank
    pool tiles (bufs=2 each): ACT activation(Relu, scale=rny, accum_out)
    / DVE tensor_scalar(max 0, accum_out) -> R, outputs to SBUF dumps
  - diag: GpSimd squares mid-loop + late DVE reduces, tiny fp32 ops
Host combines [128,2] partials; diag col used only from owner cores.
"""

import numpy as np

import concourse.bacc as bacc
import concourse.bass as bass
import concourse.tile as tile
from concourse import mybir
from concourse.bass_utils import run_bass_kernel_spmd
from concourse.masks import make_identity

N, D = 8192, 128
NCORES = 8
XI = 2048            # x rows per core
YJ = 4096            # y rows per core
TXI = XI // 128      # 16 x tiles
TYJ = YJ // 128      # 32 y j-tiles
YCH = 4              # y cast-DMA chunks
YCT = TYJ // YCH     # 8 j-tiles per chunk
ACW = 1024           # ACT's share of each [128, 2048] tile (bank aligned)

f32 = mybir.dt.float32
bf16 = mybir.dt.bfloat16
AF = mybir.ActivationFunctionType
ALU = mybir.AluOpType
AX = mybir.AxisListType

_CACHE = {}


def _build():
    if "nc" in _CACHE:
        return _CACHE["nc"]
    nc = bacc.Bacc("TRN2", target_bir_lowering=False, debug=False,
                   num_devices=NCORES)
    xs_d = nc.dram_tensor("xs", [XI, D], f32, kind="ExternalInput")
    y_d = nc.dram_tensor("y", [YJ, D], f32, kind="ExternalInput")
    yd_d = nc.dram_tensor("yd", [XI, D], f32, kind="ExternalInput")
    out_d = nc.dram_tensor("out", [128, 2], f32, kind="ExternalOutput")

    with tile.TileContext(nc) as tc:
        with (
            tc.tile_pool(name="singles", bufs=1) as singles,
            tc.tile_pool(name="scr", bufs=2) as scr,
        ):
            ident = singles.tile([128, 128], bf16)
            make_identity(nc, ident[:])
            warm = singles.tile([128, 1], f32)
            nc.vector.memset(warm[:], 1.0)
            nc.scalar.sqrt(warm[:], warm[:])   # preload sqrt table set early

            ynat = singles.tile([128, TYJ, 128], bf16)   # row j: 1024g+16p+t
            yT = singles.tile([128, TYJ, 128], bf16)     # [d, t, j-col]
            xnat = singles.tile([128, TXI, 128], f32)    # row i: 16p+8u+t
            xhat = singles.tile([128, TXI, 128], bf16)
            xhatT = singles.tile([128, TXI * 128], bf16)
            ydn = singles.tile([128, TXI, 128], bf16)    # same layout as x

            ny2 = singles.tile([128, TYJ], f32)
            t2y = singles.tile([128, TYJ], f32)
            rny = singles.tile([128, TYJ], f32)
            nx2 = singles.tile([128, TXI], f32)
            t1x = singles.tile([128, TXI], f32)
            rnx = singles.tile([128, TXI], f32)
            d2 = singles.tile([128, TXI], f32)
            nyd2 = singles.tile([128, TXI], f32)
            rnyd = singles.tile([128, TXI], f32)
            sim_d = singles.tile([128, TXI], f32)
            relu_d = singles.tile([128, TXI], f32)
            R = singles.tile([128, 64], f32)
            outsb = singles.tile([128, 2], f32)
            dumpA = singles.tile([128, ACW], bf16)
            dumpD = singles.tile([128, 2048 - ACW], bf16)

            # ---- input DMAs: x on HWDGE (fp32); y, yd on SWDGE (cast)
            xv = xs_d[:].rearrange("(p u t) d -> u p t d", u=2, t=8)
            for h in range(2):
                nc.sync.dma_start(
                    out=xnat[:, 8 * h:8 * (h + 1), :],
                    in_=xv[h:h + 1].rearrange("u p t d -> (u p) t d"))
            for g in range(YCH):
                rows = slice(1024 * g, 1024 * (g + 1))
                nc.gpsimd.dma_start(
                    out=ynat[:, YCT * g:YCT * (g + 1), :],
                    in_=y_d[rows].rearrange("(p t) d -> p t d", t=YCT))
            ydv = yd_d[:].rearrange("(p u t) d -> u p t d", u=2, t=8)
            for h in range(2):
                nc.gpsimd.dma_start(
                    out=ydn[:, 8 * h:8 * (h + 1), :],
                    in_=ydv[h:h + 1].rearrange("u p t d -> (u p) t d"))

            # ---- x norms + scale per half: ACT square, DVE reduce/scale
            for h in range(2):
                hs = slice(8 * h, 8 * (h + 1))
                xsq = scr.tile([128, 8, 128], bf16, tag="xsq",
                               name=f"xsq{h}")
                with nc.allow_low_precision("norm squares in bf16"):
                    nc.scalar.activation(
                        xsq[:].rearrange("p a b -> p (a b)"),
                        xnat[:, hs, :].rearrange("p a b -> p (a b)"),
                        AF.Square)
                nc.vector.tensor_reduce(out=nx2[:, hs], in_=xsq[:],
                                        axis=AX.X, op=ALU.add)
                nc.vector.reciprocal(t1x[:, hs], nx2[:, hs])
                nc.scalar.sqrt(rnx[:, hs], t1x[:, hs])   # 1/||x_i||
                for t in range(8 * h, 8 * h + 8):
                    nc.vector.tensor_scalar(
                        out=xhat[:, t, :], in0=xnat[:, t, :],
                        scalar1=rnx[:, t:t + 1], scalar2=None,
                        op0=ALU.mult)

            # ---- y norms per quarter: DVE single-port square + reduce
            def y_norm_tail(g):
                gs = slice(YCT * g, YCT * (g + 1))
                with nc.allow_low_precision("norm squares in bf16"):
                    s = scr.tile([128, YCT, 128], bf16, tag="ysq",
                                 name=f"ysq{g}")
                    nc.vector.tensor_mul(
                        s[:].rearrange("p a b -> p (a b)"),
                        ynat[:, gs, :].rearrange("p a b -> p (a b)"),
                        ynat[:, gs, :].rearrange("p a b -> p (a b)"))
                nc.vector.tensor_reduce(out=ny2[:, gs], in_=s[:],
                                        axis=AX.X, op=ALU.add)
                nc.vector.reciprocal(t2y[:, gs], ny2[:, gs])
                nc.scalar.sqrt(rny[:, gs], t2y[:, gs])   # 1/||y_j||

            y_norm_tail(0)
            y_norm_tail(1)

            # ---- transposes on PE (bf16) + copies to SBUF
            with tc.tile_pool(name="tpsum", bufs=2, space="PSUM") as tpsum:
                for h in range(2):
                    ptx = tpsum.tile([128, 1024], bf16, tag="tp2")
                    for k in range(8):
                        t = 8 * h + k
                        nc.tensor.transpose(ptx[:, 128 * k:128 * (k + 1)],
                                            xhat[:, t, :], ident[:])
                    nc.scalar.copy(out=xhatT[:, 1024 * h:1024 * (h + 1)],
                                   in_=ptx[:])
                for g in range(YCH):
                    pty = tpsum.tile([128, 1024], bf16, tag="tp2")
                    for k in range(YCT):
                        t = YCT * g + k
                        nc.tensor.transpose(pty[:, 128 * k:128 * (k + 1)],
                                            ynat[:, t, :], ident[:])
                    nc.vector.tensor_copy(
                        out=yT[:, YCT * g:YCT * (g + 1), :]
                        .rearrange("p a b -> p (a b)"),
                        in_=pty[:])

            # ---- main loop
            with (
                tc.tile_pool(name="mpa", bufs=2, space="PSUM") as mpa,
                tc.tile_pool(name="mpd", bufs=2, space="PSUM") as mpd,
            ):
                prodn = ydsqn = None
                for t in range(TYJ):
                    if t == 2:
                        y_norm_tail(2)
                    if t == 26:
                        # diag squares on GpSimd (idle mid-loop, SBUF only)
                        prodn = scr.tile([128, TXI, 128], bf16, tag="gp",
                                         name="prodn")
                        nc.gpsimd.tensor_mul(
                            prodn[:].rearrange("p a b -> p (a b)"),
                            xhat[:].rearrange("p a b -> p (a b)"),
                            ydn[:].rearrange("p a b -> p (a b)"))
                        ydsqn = scr.tile([128, TXI, 128], bf16, tag="gq",
                                         name="ydsqn")
                        nc.gpsimd.tensor_mul(
                            ydsqn[:].rearrange("p a b -> p (a b)"),
                            ydn[:].rearrange("p a b -> p (a b)"),
                            ydn[:].rearrange("p a b -> p (a b)"))
                    if t == 10:
                        y_norm_tail(3)
                    if t == 30:
                        nc.vector.tensor_reduce(out=d2[:], in_=prodn[:],
                                                axis=AX.X, op=ALU.add)
                    if t == 31:
                        nc.vector.tensor_reduce(out=nyd2[:], in_=ydsqn[:],
                                                axis=AX.X, op=ALU.add)

                    lhsT = yT[:, t, :]
                    pa = mpa.tile([128, ACW], f32, tag="pa")
                    pd = mpd.tile([128, 2048 - ACW], f32, tag="pd")
                    for k in range(4):
                        col = 512 * k
                        dst = (pa[:, col:col + 512] if col < ACW
                               else pd[:, col - ACW:col - ACW + 512])
                        nc.tensor.matmul(dst, lhsT,
                                         xhatT[:, col:col + 512])
                    nc.scalar.activation(
                        dumpA[:], pa[:], AF.Relu,
                        scale=rny[:, t:t + 1],
                        accum_out=R[:, 2 * t:2 * t + 1])
                    nc.vector.tensor_scalar(
                        out=dumpD[:], in0=pd[:],
                        scalar1=0.0, scalar2=None,
                        op0=ALU.max, op1=ALU.add,
                        accum_out=R[:, 2 * t + 1:2 * t + 2])

            # post-scale DVE R columns (odd) by rny
            nc.vector.tensor_mul(R[:, 1:64:2], R[:, 1:64:2], rny[:, 0:TYJ])

            # ---- diag scalars
            nc.vector.reciprocal(t1x[:], nyd2[:])
            nc.scalar.sqrt(rnyd[:], t1x[:])
            nc.vector.tensor_mul(sim_d[:], d2[:], rnyd[:])
            nc.scalar.activation(relu_d[:], sim_d[:], AF.Relu)
            nc.vector.scalar_tensor_tensor(
                out=scr.tile([128, TXI], f32, tag="dd", name="dd")[:],
                in0=sim_d[:], scalar=1.0, in1=relu_d[:],
                op0=ALU.mult, op1=ALU.add, accum_out=outsb[:, 1:2])

            # ---- final: sum R columns
            nc.vector.tensor_reduce(out=outsb[:, 0:1], in_=R[:],
                                    axis=AX.X, op=ALU.add)
            nc.sync.dma_start(out=out_d[:], in_=outsb[:])

    nc.compile()
    _CACHE["nc"] = nc
    return nc


# cores whose x block lies inside their y range own the diag correction
_DIAG_OWNER = [1, 0, 1, 0, 0, 1, 0, 1]


def _in_maps(x, y):
    maps = []
    for c in range(NCORES):
        bi, bj = c // 2, c % 2
        xsl = slice(XI * bi, XI * (bi + 1))
        ysl = slice(YJ * bj, YJ * (bj + 1))
        maps.append({"xs": np.ascontiguousarray(x[xsl]),
                     "y": np.ascontiguousarray(y[ysl]),
                     "yd": np.ascontiguousarray(y[xsl])})
    return maps


def _combine(results):
    total = 0.0
    for c in range(NCORES):
        o = results[c]["out"].astype(np.float64)
        total += o[:, 0].sum()
        if _DIAG_OWNER[c]:
            total += XI - o[:, 1].sum()
    return np.float32(total / (float(N) * float(N)))


def _run(x, y, trace=False):
    nc = _build()
    res = run_bass_kernel_spmd(nc, _in_maps(x, y), list(range(NCORES)),
                               trace=trace)
    return _combine(res.results), res


def kernel(x, y):
    x = np.asarray(x, dtype=np.float32)
    y = np.asarray(y, dtype=np.float32)
    loss, _ = _run(x, y, trace=False)
    return loss
